# revision 34
# baseline (speedup 1.0000x reference)
"""TRN2 Bass kernel for nn_LogDomainResNet.

The reference network is a signed-log-domain encoding of a plain
real-domain tanh ResNet:

    v0      = sign_x * exp(log_abs_x)
    v_{i+1} = tanh(v_i @ W_i + b_i) + v_i        (7 inner layers)
    t       = v_7 @ W_final
    out     = stack([sign(t), log|t|])

All slog plumbing (per-row max, exp/log per layer) cancels exactly, so the
kernel computes in the real domain. Values stay bounded (|v| < 24), so fp32
range is never an issue.

Precision: each matmul is one fp16 "main" pass plus one fp8 DoubleRow
"correction" pass. With x ~ xh16 + xl and W ~ wh16 + wl, the correction
computes xl@W + xh@wl in a single DoubleRow instruction pair
(slot0 = xl8*2^9 . wh8*2^4, slot1 = xh8*2^2 . wl8*2^11 - both products
carry 2^13, and the main pass's fp16 weights are pre-scaled by 2^13 so
all passes share one PSUM accumulation group). The 2^-13 unscale rides
the ACT engine's activation scale for free. Net ~2.1 bf16-rate passes
per matmul instead of 3.

Layout: activations live transposed ([feature -> partitions, batch -> free])
so the per-output-feature bias rides the ACT engine's per-partition bias and
the weight chunks feed the PE as the stationary operand directly. The final
layer swaps operands (lhsT = v^T tile) to produce t in natural [batch,
feature] layout, so outputs DMA out contiguously with no transposes.

Sharding: data-parallel over the batch axis, 1024 rows per core x 8 cores.
"""

import numpy as np

_B, _D, _NL = 8192, 1024, 8  # batch, width, layers (7 inner + final)
_NCORES = 8
_BP = _B // _NCORES          # batch rows per core
_P = 128
_KC = _D // _P               # contraction chunks per matmul
_BT = _BP // _P              # batch tiles (input/final stages)
_BCH = 512                   # PSUM free dim
_NBC = _BP // _BCH           # batch chunks per layer pass
_NT = _D // _P               # out-feature tiles per layer

_SM = 2.0 ** 13              # shared PSUM scale of main+correction passes
_SXL = 2.0 ** 9              # xl8 storage scale
_SXH = 2.0 ** 2              # xh8 storage scale
_SWH = 2.0 ** 4              # wh8 storage scale
_SWL = 2.0 ** 11             # wl8 storage scale

_cached_nc = None
last_results = None  # BassKernelResults from the most recent run (for test.py)


def _build():
    import concourse.mybir as mybir
    from concourse import bacc
    from concourse.tile import TileContext
    from concourse.masks import make_identity

    f32, f16 = mybir.dt.float32, mybir.dt.float16
    bf16, f8 = mybir.dt.bfloat16, mybir.dt.float8e4
    AF = mybir.ActivationFunctionType
    DR = mybir.MatmulPerfMode.DoubleRow

    nc = bacc.Bacc("TRN2", target_bir_lowering=False, debug=False)
    d_sgn = nc.dram_tensor("sign_x", [_BP, _D], f8, kind="ExternalInput")
    d_lab = nc.dram_tensor("log_abs_x", [_BP, _D], f32, kind="ExternalInput")
    d_wh = nc.dram_tensor("wh", [_NL, _D, _D], f16, kind="ExternalInput")
    d_w8 = nc.dram_tensor("w8", [_NL * 2, _D, _D], f8, kind="ExternalInput")
    d_bias = nc.dram_tensor("bias", [_NL - 1, _D], f32, kind="ExternalInput")
    d_outs = nc.dram_tensor("out_s", [_BP, _D], f8, kind="ExternalOutput")
    d_outl = nc.dram_tensor("out_l", [_BP, _D], f16, kind="ExternalOutput")

    with TileContext(nc) as tc:
        with (
            tc.tile_pool(name="const", bufs=1) as constp,
            tc.tile_pool(name="w", bufs=2) as wp,
            tc.tile_pool(name="v", bufs=2) as vp,
            tc.tile_pool(name="inp", bufs=4) as inp,
            tc.tile_pool(name="tmp", bufs=3) as tmp,
            tc.tile_pool(name="fin", bufs=2) as fin,
            tc.tile_pool(name="ps", bufs=4, space="PSUM") as ps,
            tc.tile_pool(name="pst", bufs=3, space="PSUM") as pst,
        ):
            ident = constp.tile([_P, _P], f32)
            make_identity(nc, ident[:])
            bias_sb = constp.tile([_P, (_NL - 1) * _NT], f32)
            nc.sync.dma_start(
                bias_sb[:], d_bias.rearrange("l (t p) -> p (l t)", p=_P)
            )

            # ---- input: v0 = sign * exp(log_abs), PE-transposed to vT ----
            # Activation tensors are split per batch-half so that layer i+1's
            # first-half matmuls depend only on layer i's first-half writes
            # (avoids a whole-tile dependency stall at every layer boundary).
            vh = [vp.tile([_P, _KC, _BCH], f16, tag=f"vh{h}", name=f"vh{h}") for h in range(_NBC)]
            vl = [vp.tile([_P, _KC, _BCH], bf16, tag=f"vl{h}", name=f"vl{h}") for h in range(_NBC)]
            v8 = [vp.tile([_P, _KC, 2, _BCH], f8, tag=f"v8{h}", name=f"v8{h}") for h in range(_NBC)]
            def emit_input_load(t):
                lab_t = inp.tile([_P, _D], f32, tag="lab", name="lab_t")
                nc.sync.dma_start(lab_t[:], d_lab[t * _P : (t + 1) * _P, :])
                sgn_t = inp.tile([_P, _D], f8, tag="sgn", name="sgn_t")
                nc.sync.dma_start(sgn_t[:], d_sgn[t * _P : (t + 1) * _P, :])
                nc.scalar.activation(lab_t[:], lab_t[:], AF.Exp)
                nc.vector.tensor_mul(out=lab_t[:], in0=lab_t[:], in1=sgn_t[:])
                return lab_t

            def emit_input_split_half(t, lab_t, g):
                # transpose 4 chunks into one wide PSUM tile, then wide splits
                h, hoff = divmod(t * _P, _BCH)
                tsl = slice(hoff, hoff + _P)
                csl = slice(g * 4, g * 4 + 4)
                ptile = pst.tile([_P, 4, _P], f32, tag="tr", name="ptile")
                for k in range(4):
                    nc.tensor.transpose(
                        ptile[:, k, :],
                        lab_t[:, (g * 4 + k) * _P : (g * 4 + k + 1) * _P],
                        ident[:],
                    )
                nc.scalar.activation(vh[h][:, csl, tsl], ptile[:], AF.Copy)
                nc.vector.tensor_sub(
                    out=vl[h][:, csl, tsl], in0=ptile[:], in1=vh[h][:, csl, tsl]
                )
                nc.vector.tensor_scalar_mul(
                    v8[h][:, csl, 0, tsl], vl[h][:, csl, tsl], _SXL
                )
                nc.scalar.activation(
                    v8[h][:, csl, 1, tsl], ptile[:], AF.Copy, scale=_SXH
                )

            def emit_input_tile(t):
                lab_t = emit_input_load(t)
                emit_input_split_half(t, lab_t, 0)
                emit_input_split_half(t, lab_t, 1)

            def emit_layer_weights(i):
                # Quarter-DMAs for the fp16 weights (512B DRAM segments, and
                # the first matmul groups only wait for their own quarter);
                # whole-plane DMAs for the fp8 pair (1KB segments).
                whs = wp.tile([_P, _KC, _D], f16, tag="wh", name="whs")
                w8s = wp.tile([_P, _KC, 2, _D], f8, tag="w8", name="w8s")
                src_h = d_wh[i].rearrange("(c p) n -> p c n", p=_P)
                src_8h = d_w8[2 * i].rearrange("(c p) n -> p c n", p=_P)
                src_8l = d_w8[2 * i + 1].rearrange("(c p) n -> p c n", p=_P)
                for q in range(4):
                    qsl = slice(q * (_D // 4), (q + 1) * (_D // 4))
                    nc.sync.dma_start(whs[:, :, qsl], src_h[:, :, qsl])
                nc.sync.dma_start(w8s[:, :, 0, :], src_8h)
                nc.sync.dma_start(w8s[:, :, 1, :], src_8l)
                return whs, w8s

            def emit_layer_tile(i, bc, n, whs, w8s, vhn, vln, v8n):
                nsl = slice(n * _P, (n + 1) * _P)
                pt = ps.tile([_P, _BCH], f32, tag="mm", name="pt")
                for c in range(_KC):
                    nc.tensor.matmul(
                        pt[:], whs[:, c, nsl], vh[bc][:, c, :],
                        start=(c == 0), stop=False,
                    )
                for c in range(_KC):
                    nc.tensor.matmul(
                        pt[:], w8s[:, c, :, nsl], v8[bc][:, c, :, :],
                        start=False, stop=(c == _KC - 1),
                        perf_mode=DR,
                    )
                u = tmp.tile([_P, _BCH], f32, tag="u", name="u")
                nc.scalar.activation(
                    u[:], pt[:], AF.Tanh,
                    bias=bias_sb[:, i * _NT + n : i * _NT + n + 1],
                    scale=1.0 / _SM,
                )
                a = tmp.tile([_P, _BCH], f32, tag="a", name="a")
                nc.vector.tensor_add(out=a[:], in0=u[:], in1=vh[bc][:, n, :])
                nc.vector.tensor_add(out=a[:], in0=a[:], in1=vl[bc][:, n, :])
                nc.scalar.activation(vhn[bc][:, n, :], a[:], AF.Copy)
                nc.vector.tensor_sub(
                    out=vln[bc][:, n, :], in0=a[:], in1=vhn[bc][:, n, :]
                )
                nc.vector.tensor_scalar_mul(
                    v8n[bc][:, n, 0, :], vln[bc][:, n, :], _SXL
                )
                nc.scalar.activation(
                    v8n[bc][:, n, 1, :], a[:], AF.Copy, scale=_SXH
                )

            def new_v_tiles():
                vhn = [
                    vp.tile([_P, _KC, _BCH], f16, tag=f"vh{h}", name=f"vhn{h}")
                    for h in range(_NBC)
                ]
                vln = [
                    vp.tile([_P, _KC, _BCH], bf16, tag=f"vl{h}", name=f"vln{h}")
                    for h in range(_NBC)
                ]
                v8n = [
                    vp.tile([_P, _KC, 2, _BCH], f8, tag=f"v8{h}", name=f"v8n{h}")
                    for h in range(_NBC)
                ]
                return vhn, vln, v8n

            # ---- layer 0, interleaved with second-half input processing ----
            for t in range(_BT // 2):
                emit_input_tile(t)
            # layer-0 weights, finely interleaved with the second-half input
            # loads so the first matmul groups and the input pipeline are
            # both fed as early as possible.
            whs = wp.tile([_P, _KC, _D], f16, tag="wh", name="whs")
            w8s = wp.tile([_P, _KC, 2, _D], f8, tag="w8", name="w8s")
            src_h = d_wh[0].rearrange("(c p) n -> p c n", p=_P)
            src_8h = d_w8[0].rearrange("(c p) n -> p c n", p=_P)
            src_8l = d_w8[1].rearrange("(c p) n -> p c n", p=_P)
            Q = _D // 4
            nc.sync.dma_start(whs[:, :, 0:Q], src_h[:, :, 0:Q])
            nc.sync.dma_start(w8s[:, :, 0, 0 : 2 * Q], src_8h[:, :, 0 : 2 * Q])
            nc.sync.dma_start(w8s[:, :, 1, 0 : 2 * Q], src_8l[:, :, 0 : 2 * Q])
            labs = [emit_input_load(_BT // 2 + k) for k in range(_BT // 2)]
            for q in range(1, 4):
                nc.sync.dma_start(
                    whs[:, :, q * Q : (q + 1) * Q], src_h[:, :, q * Q : (q + 1) * Q]
                )
            nc.sync.dma_start(w8s[:, :, 0, 2 * Q :], src_8h[:, :, 2 * Q :])
            nc.sync.dma_start(w8s[:, :, 1, 2 * Q :], src_8l[:, :, 2 * Q :])
            vhn, vln, v8n = new_v_tiles()
            halves = [(_BT // 2 + k // 2, k % 2) for k in range(_BT)]
            for n in range(_NT):
                emit_layer_tile(0, 0, n, whs, w8s, vhn, vln, v8n)
                if n >= 1:
                    t, g = halves[n - 1]
                    emit_input_split_half(t, labs[t - _BT // 2], g)
            t, g = halves[_NT - 1]
            emit_input_split_half(t, labs[t - _BT // 2], g)
            for n in range(_NT):
                emit_layer_tile(0, 1, n, whs, w8s, vhn, vln, v8n)
            vh, vl, v8 = vhn, vln, v8n

            # ---- inner layers 1..6: v = tanh(v @ W + b) + v ----
            # Weights for layer i+1 are DMA'd from the middle of layer i-1's
            # stream (the wp pool buffer is free by then), so each layer's
            # weights have a full layer span of DMA lead time.
            next_w = emit_layer_weights(1)
            for i in range(1, _NL - 1):
                whs, w8s = next_w
                next_w = None
                vhn, vln, v8n = new_v_tiles()
                for bc in range(_NBC):
                    for n in range(_NT):
                        emit_layer_tile(i, bc, n, whs, w8s, vhn, vln, v8n)
                        if bc == 0 and n == 1:
                            next_w = emit_layer_weights(i + 1)
                vh, vl, v8 = vhn, vln, v8n

            # ---- final layer: t = v @ W_f, out = [sign(t), log|t|] ----
            whf, w8f = next_w
            for bt in range(_BT):
                h, hoff = divmod(bt * _P, _BCH)
                bsl = slice(hoff, hoff + _P)
                for j in range(_NBC):
                    nsl = slice(j * _BCH, (j + 1) * _BCH)
                    pt = ps.tile([_P, _BCH], f32, tag="mm")
                    for c in range(_KC):
                        nc.tensor.matmul(
                            pt[:], vh[h][:, c, bsl], whf[:, c, nsl],
                            start=(c == 0), stop=False,
                        )
                    for c in range(_KC):
                        nc.tensor.matmul(
                            pt[:], v8[h][:, c, :, bsl], w8f[:, c, :, nsl],
                            start=False, stop=(c == _KC - 1),
                            perf_mode=DR,
                        )
                    sg = fin.tile([_P, _BCH], f8, tag="sg")
                    nc.scalar.activation(sg[:], pt[:], AF.Sign)
                    ab = fin.tile([_P, _BCH], f32, tag="ab")
                    nc.scalar.activation(ab[:], pt[:], AF.Abs, scale=1.0 / _SM)
                    lg = fin.tile([_P, _BCH], f16, tag="lg")
                    nc.scalar.activation(lg[:], ab[:], AF.Ln)
                    osl = slice(bt * _P, (bt + 1) * _P)
                    nc.sync.dma_start(d_outs[osl, nsl], sg[:])
                    nc.sync.dma_start(d_outl[osl, nsl], lg[:])
    nc.compile()
    return nc


def kernel(sign_x, log_abs_x, inner_kernels, final_kernel):
    global _cached_nc, last_results
    import ml_dtypes
    from concourse.bass_utils import run_bass_kernel_spmd

    if _cached_nc is None:
        _cached_nc = _build()
    nc = _cached_nc

    sign_x = np.ascontiguousarray(
        np.asarray(sign_x, dtype=np.float32).astype(ml_dtypes.float8_e4m3)
    )
    log_abs_x = np.ascontiguousarray(np.asarray(log_abs_x, dtype=np.float32))
    ik = np.asarray(inner_kernels, dtype=np.float32)
    fk = np.asarray(final_kernel, dtype=np.float32)

    E4 = ml_dtypes.float8_e4m3  # TRN FP8_EXP4: max finite +-240

    def q4(x):
        return np.clip(x, -240.0, 240.0).astype(E4)

    W = np.concatenate([ik[:, :_D, :], fk[None]], axis=0)  # [8, 1024, 1024]
    wh16 = W.astype(np.float16)
    wl = W - wh16.astype(np.float32)
    Wh = np.ascontiguousarray((wh16.astype(np.float32) * _SM).astype(np.float16))
    W8 = np.ascontiguousarray(
        np.stack([q4(W * _SWH), q4(wl * _SWL)], axis=1)  # [8, 2, 1024, 1024]
    )
    bias = np.ascontiguousarray(ik[:, _D, :])  # [7, 1024]

    in_maps = []
    for cid in range(_NCORES):
        sl = slice(cid * _BP, (cid + 1) * _BP)
        in_maps.append({
            "sign_x": np.ascontiguousarray(sign_x[sl]),
            "log_abs_x": np.ascontiguousarray(log_abs_x[sl]),
            "wh": Wh,
            "w8": W8,
            "bias": bias,
        })

    last_results = run_bass_kernel_spmd(nc, in_maps, core_ids=list(range(_NCORES)))
    out = np.empty((2, _B, _D), dtype=np.float32)
    for cid, r in enumerate(last_results.results):
        sl = slice(cid * _BP, (cid + 1) * _BP)
        out[0, sl] = r["out_s"].astype(np.float32)
        out[1, sl] = r["out_l"].astype(np.float32)
    return out


# revision 35
# speedup vs baseline: 1.0038x; 1.0038x over previous
"""TRN2 Bass kernel for nn_LogDomainResNet.

The reference network is a signed-log-domain encoding of a plain
real-domain tanh ResNet:

    v0      = sign_x * exp(log_abs_x)
    v_{i+1} = tanh(v_i @ W_i + b_i) + v_i        (7 inner layers)
    t       = v_7 @ W_final
    out     = stack([sign(t), log|t|])

All slog plumbing (per-row max, exp/log per layer) cancels exactly, so the
kernel computes in the real domain. Values stay bounded (|v| < 24), so fp32
range is never an issue.

Precision: each matmul is one fp16 "main" pass plus one fp8 DoubleRow
"correction" pass. With x ~ xh16 + xl and W ~ wh16 + wl, the correction
computes xl@W + xh@wl in a single DoubleRow instruction pair
(slot0 = xl8*2^9 . wh8*2^4, slot1 = xh8*2^2 . wl8*2^11 - both products
carry 2^13, and the main pass's fp16 weights are pre-scaled by 2^13 so
all passes share one PSUM accumulation group). The 2^-13 unscale rides
the ACT engine's activation scale for free. Net ~2.1 bf16-rate passes
per matmul instead of 3.

Layout: activations live transposed ([feature -> partitions, batch -> free])
so the per-output-feature bias rides the ACT engine's per-partition bias and
the weight chunks feed the PE as the stationary operand directly. The final
layer swaps operands (lhsT = v^T tile) to produce t in natural [batch,
feature] layout, so outputs DMA out contiguously with no transposes.

Sharding: data-parallel over the batch axis, 1024 rows per core x 8 cores.
"""

import numpy as np

_B, _D, _NL = 8192, 1024, 8  # batch, width, layers (7 inner + final)
_NCORES = 8
_BP = _B // _NCORES          # batch rows per core
_P = 128
_KC = _D // _P               # contraction chunks per matmul
_BT = _BP // _P              # batch tiles (input/final stages)
_BCH = 512                   # PSUM free dim
_NBC = _BP // _BCH           # batch chunks per layer pass
_NT = _D // _P               # out-feature tiles per layer

_SM = 2.0 ** 13              # shared PSUM scale of main+correction passes
_SXL = 2.0 ** 9              # xl8 storage scale
_SXH = 2.0 ** 2              # xh8 storage scale
_SWH = 2.0 ** 4              # wh8 storage scale
_SWL = 2.0 ** 11             # wl8 storage scale

_cached_nc = None
last_results = None  # BassKernelResults from the most recent run (for test.py)


def _build():
    import concourse.mybir as mybir
    from concourse import bacc
    from concourse.tile import TileContext
    from concourse.masks import make_identity

    f32, f16 = mybir.dt.float32, mybir.dt.float16
    bf16, f8 = mybir.dt.bfloat16, mybir.dt.float8e4
    AF = mybir.ActivationFunctionType
    DR = mybir.MatmulPerfMode.DoubleRow

    nc = bacc.Bacc("TRN2", target_bir_lowering=False, debug=False)
    d_sgn = nc.dram_tensor("sign_x", [_BP, _D], f8, kind="ExternalInput")
    d_lab = nc.dram_tensor("log_abs_x", [_BP, _D], f32, kind="ExternalInput")
    d_wh = nc.dram_tensor("wh", [_NL, _D, _D], f16, kind="ExternalInput")
    d_w8 = nc.dram_tensor("w8", [_NL * 2, _D, _D], f8, kind="ExternalInput")
    d_bias = nc.dram_tensor("bias", [_NL - 1, _D], f32, kind="ExternalInput")
    d_outs = nc.dram_tensor("out_s", [_BP, _D], f8, kind="ExternalOutput")
    d_outl = nc.dram_tensor("out_l", [_BP, _D], f16, kind="ExternalOutput")

    with TileContext(nc) as tc:
        with (
            tc.tile_pool(name="const", bufs=1) as constp,
            tc.tile_pool(name="w", bufs=2) as wp,
            tc.tile_pool(name="v", bufs=2) as vp,
            tc.tile_pool(name="inp", bufs=4) as inp,
            tc.tile_pool(name="tmp", bufs=3) as tmp,
            tc.tile_pool(name="fin", bufs=2) as fin,
            tc.tile_pool(name="ps", bufs=4, space="PSUM") as ps,
            tc.tile_pool(name="pst", bufs=3, space="PSUM") as pst,
        ):
            ident = constp.tile([_P, _P], f32)
            make_identity(nc, ident[:])
            bias_sb = constp.tile([_P, (_NL - 1) * _NT], f32)
            nc.sync.dma_start(
                bias_sb[:], d_bias.rearrange("l (t p) -> p (l t)", p=_P)
            )

            # ---- input: v0 = sign * exp(log_abs), PE-transposed to vT ----
            # Activation tensors are split per batch-half so that layer i+1's
            # first-half matmuls depend only on layer i's first-half writes
            # (avoids a whole-tile dependency stall at every layer boundary).
            vh = [vp.tile([_P, _KC, _BCH], f16, tag=f"vh{h}", name=f"vh{h}") for h in range(_NBC)]
            vl = [vp.tile([_P, _KC, _BCH], bf16, tag=f"vl{h}", name=f"vl{h}") for h in range(_NBC)]
            v8 = [vp.tile([_P, _KC, 2, _BCH], f8, tag=f"v8{h}", name=f"v8{h}") for h in range(_NBC)]
            def emit_input_load(t):
                lab_t = inp.tile([_P, _D], f32, tag="lab", name="lab_t")
                nc.sync.dma_start(lab_t[:], d_lab[t * _P : (t + 1) * _P, :])
                sgn_t = inp.tile([_P, _D], f8, tag="sgn", name="sgn_t")
                nc.sync.dma_start(sgn_t[:], d_sgn[t * _P : (t + 1) * _P, :])
                nc.scalar.activation(lab_t[:], lab_t[:], AF.Exp)
                nc.vector.tensor_mul(out=lab_t[:], in0=lab_t[:], in1=sgn_t[:])
                return lab_t

            def emit_input_split_half(t, lab_t, g):
                # transpose 4 chunks into one wide PSUM tile, then wide splits
                h, hoff = divmod(t * _P, _BCH)
                tsl = slice(hoff, hoff + _P)
                csl = slice(g * 4, g * 4 + 4)
                ptile = pst.tile([_P, 4, _P], f32, tag="tr", name="ptile")
                for k in range(4):
                    nc.tensor.transpose(
                        ptile[:, k, :],
                        lab_t[:, (g * 4 + k) * _P : (g * 4 + k + 1) * _P],
                        ident[:],
                    )
                nc.scalar.activation(vh[h][:, csl, tsl], ptile[:], AF.Copy)
                nc.vector.tensor_sub(
                    out=vl[h][:, csl, tsl], in0=ptile[:], in1=vh[h][:, csl, tsl]
                )
                nc.vector.tensor_scalar_mul(
                    v8[h][:, csl, 0, tsl], vl[h][:, csl, tsl], _SXL
                )
                nc.scalar.activation(
                    v8[h][:, csl, 1, tsl], ptile[:], AF.Copy, scale=_SXH
                )

            def emit_input_tile(t):
                lab_t = emit_input_load(t)
                emit_input_split_half(t, lab_t, 0)
                emit_input_split_half(t, lab_t, 1)

            def emit_layer_weights(i):
                # Quarter-DMAs for the fp16 weights (512B DRAM segments, and
                # the first matmul groups only wait for their own quarter);
                # whole-plane DMAs for the fp8 pair (1KB segments).
                whs = wp.tile([_P, _KC, _D], f16, tag="wh", name="whs")
                w8s = wp.tile([_P, _KC, 2, _D], f8, tag="w8", name="w8s")
                src_h = d_wh[i].rearrange("(c p) n -> p c n", p=_P)
                src_8h = d_w8[2 * i].rearrange("(c p) n -> p c n", p=_P)
                src_8l = d_w8[2 * i + 1].rearrange("(c p) n -> p c n", p=_P)
                for q in range(4):
                    qsl = slice(q * (_D // 4), (q + 1) * (_D // 4))
                    nc.sync.dma_start(whs[:, :, qsl], src_h[:, :, qsl])
                nc.sync.dma_start(w8s[:, :, 0, :], src_8h)
                nc.sync.dma_start(w8s[:, :, 1, :], src_8l)
                return whs, w8s

            def emit_layer_tile(i, bc, n, whs, w8s, vhn, vln, v8n):
                nsl = slice(n * _P, (n + 1) * _P)
                pt = ps.tile([_P, _BCH], f32, tag="mm", name="pt")
                for c in range(_KC):
                    nc.tensor.matmul(
                        pt[:], whs[:, c, nsl], vh[bc][:, c, :],
                        start=(c == 0), stop=False,
                    )
                for c in range(_KC):
                    nc.tensor.matmul(
                        pt[:], w8s[:, c, :, nsl], v8[bc][:, c, :, :],
                        start=False, stop=(c == _KC - 1),
                        perf_mode=DR,
                    )
                u = tmp.tile([_P, _BCH], f32, tag="u", name="u")
                nc.scalar.activation(
                    u[:], pt[:], AF.Tanh,
                    bias=bias_sb[:, i * _NT + n : i * _NT + n + 1],
                    scale=1.0 / _SM,
                )
                a = tmp.tile([_P, _BCH], f32, tag="a", name="a")
                nc.vector.tensor_add(out=a[:], in0=u[:], in1=vh[bc][:, n, :])
                nc.vector.tensor_add(out=a[:], in0=a[:], in1=vl[bc][:, n, :])
                nc.scalar.activation(vhn[bc][:, n, :], a[:], AF.Copy)
                nc.vector.tensor_sub(
                    out=vln[bc][:, n, :], in0=a[:], in1=vhn[bc][:, n, :]
                )
                nc.vector.tensor_scalar_mul(
                    v8n[bc][:, n, 0, :], vln[bc][:, n, :], _SXL
                )
                nc.scalar.activation(
                    v8n[bc][:, n, 1, :], a[:], AF.Copy, scale=_SXH
                )

            def new_v_tiles():
                vhn = [
                    vp.tile([_P, _KC, _BCH], f16, tag=f"vh{h}", name=f"vhn{h}")
                    for h in range(_NBC)
                ]
                vln = [
                    vp.tile([_P, _KC, _BCH], bf16, tag=f"vl{h}", name=f"vln{h}")
                    for h in range(_NBC)
                ]
                v8n = [
                    vp.tile([_P, _KC, 2, _BCH], f8, tag=f"v8{h}", name=f"v8n{h}")
                    for h in range(_NBC)
                ]
                return vhn, vln, v8n

            # ---- layer 0, interleaved with second-half input processing ----
            for t in range(_BT // 2):
                emit_input_tile(t)
            # layer-0 weights, finely interleaved with the second-half input
            # loads so the first matmul groups and the input pipeline are
            # both fed as early as possible.
            whs = wp.tile([_P, _KC, _D], f16, tag="wh", name="whs")
            w8s = wp.tile([_P, _KC, 2, _D], f8, tag="w8", name="w8s")
            src_h = d_wh[0].rearrange("(c p) n -> p c n", p=_P)
            src_8h = d_w8[0].rearrange("(c p) n -> p c n", p=_P)
            src_8l = d_w8[1].rearrange("(c p) n -> p c n", p=_P)
            Q = _D // 4
            nc.sync.dma_start(whs[:, :, 0:Q], src_h[:, :, 0:Q])
            nc.sync.dma_start(w8s[:, :, 0, 0:Q], src_8h[:, :, 0:Q])
            nc.sync.dma_start(w8s[:, :, 1, 0:Q], src_8l[:, :, 0:Q])
            labs = [emit_input_load(_BT // 2 + k) for k in range(_BT // 2)]
            for q in range(1, 4):
                qsl = slice(q * Q, (q + 1) * Q)
                nc.sync.dma_start(whs[:, :, qsl], src_h[:, :, qsl])
                nc.sync.dma_start(w8s[:, :, 0, qsl], src_8h[:, :, qsl])
                nc.sync.dma_start(w8s[:, :, 1, qsl], src_8l[:, :, qsl])
            vhn, vln, v8n = new_v_tiles()
            halves = [(_BT // 2 + k // 2, k % 2) for k in range(_BT)]
            for n in range(_NT):
                emit_layer_tile(0, 0, n, whs, w8s, vhn, vln, v8n)
                if n >= 1:
                    t, g = halves[n - 1]
                    emit_input_split_half(t, labs[t - _BT // 2], g)
            t, g = halves[_NT - 1]
            emit_input_split_half(t, labs[t - _BT // 2], g)
            for n in range(_NT):
                emit_layer_tile(0, 1, n, whs, w8s, vhn, vln, v8n)
            vh, vl, v8 = vhn, vln, v8n

            # ---- inner layers 1..6: v = tanh(v @ W + b) + v ----
            # Weights for layer i+1 are DMA'd from the middle of layer i-1's
            # stream (the wp pool buffer is free by then), so each layer's
            # weights have a full layer span of DMA lead time.
            next_w = emit_layer_weights(1)
            for i in range(1, _NL - 1):
                whs, w8s = next_w
                next_w = None
                vhn, vln, v8n = new_v_tiles()
                for bc in range(_NBC):
                    for n in range(_NT):
                        emit_layer_tile(i, bc, n, whs, w8s, vhn, vln, v8n)
                        if bc == 0 and n == 1:
                            next_w = emit_layer_weights(i + 1)
                vh, vl, v8 = vhn, vln, v8n

            # ---- final layer: t = v @ W_f, out = [sign(t), log|t|] ----
            whf, w8f = next_w
            for bt in range(_BT):
                h, hoff = divmod(bt * _P, _BCH)
                bsl = slice(hoff, hoff + _P)
                for j in range(_NBC):
                    nsl = slice(j * _BCH, (j + 1) * _BCH)
                    pt = ps.tile([_P, _BCH], f32, tag="mm")
                    for c in range(_KC):
                        nc.tensor.matmul(
                            pt[:], vh[h][:, c, bsl], whf[:, c, nsl],
                            start=(c == 0), stop=False,
                        )
                    for c in range(_KC):
                        nc.tensor.matmul(
                            pt[:], v8[h][:, c, :, bsl], w8f[:, c, :, nsl],
                            start=False, stop=(c == _KC - 1),
                            perf_mode=DR,
                        )
                    sg = fin.tile([_P, _BCH], f8, tag="sg")
                    nc.scalar.activation(sg[:], pt[:], AF.Sign)
                    ab = fin.tile([_P, _BCH], f32, tag="ab")
                    nc.scalar.activation(ab[:], pt[:], AF.Abs, scale=1.0 / _SM)
                    lg = fin.tile([_P, _BCH], f16, tag="lg")
                    nc.scalar.activation(lg[:], ab[:], AF.Ln)
                    osl = slice(bt * _P, (bt + 1) * _P)
                    nc.sync.dma_start(d_outs[osl, nsl], sg[:])
                    nc.sync.dma_start(d_outl[osl, nsl], lg[:])
    nc.compile()
    return nc


def kernel(sign_x, log_abs_x, inner_kernels, final_kernel):
    global _cached_nc, last_results
    import ml_dtypes
    from concourse.bass_utils import run_bass_kernel_spmd

    if _cached_nc is None:
        _cached_nc = _build()
    nc = _cached_nc

    sign_x = np.ascontiguousarray(
        np.asarray(sign_x, dtype=np.float32).astype(ml_dtypes.float8_e4m3)
    )
    log_abs_x = np.ascontiguousarray(np.asarray(log_abs_x, dtype=np.float32))
    ik = np.asarray(inner_kernels, dtype=np.float32)
    fk = np.asarray(final_kernel, dtype=np.float32)

    E4 = ml_dtypes.float8_e4m3  # TRN FP8_EXP4: max finite +-240

    def q4(x):
        return np.clip(x, -240.0, 240.0).astype(E4)

    W = np.concatenate([ik[:, :_D, :], fk[None]], axis=0)  # [8, 1024, 1024]
    wh16 = W.astype(np.float16)
    wl = W - wh16.astype(np.float32)
    Wh = np.ascontiguousarray((wh16.astype(np.float32) * _SM).astype(np.float16))
    W8 = np.ascontiguousarray(
        np.stack([q4(W * _SWH), q4(wl * _SWL)], axis=1)  # [8, 2, 1024, 1024]
    )
    bias = np.ascontiguousarray(ik[:, _D, :])  # [7, 1024]

    in_maps = []
    for cid in range(_NCORES):
        sl = slice(cid * _BP, (cid + 1) * _BP)
        in_maps.append({
            "sign_x": np.ascontiguousarray(sign_x[sl]),
            "log_abs_x": np.ascontiguousarray(log_abs_x[sl]),
            "wh": Wh,
            "w8": W8,
            "bias": bias,
        })

    last_results = run_bass_kernel_spmd(nc, in_maps, core_ids=list(range(_NCORES)))
    out = np.empty((2, _B, _D), dtype=np.float32)
    for cid, r in enumerate(last_results.results):
        sl = slice(cid * _BP, (cid + 1) * _BP)
        out[0, sl] = r["out_s"].astype(np.float32)
        out[1, sl] = r["out_l"].astype(np.float32)
    return out
